# revision 3
# baseline (speedup 1.0000x reference)
"""FAGCN message-passing kernel for 8 Trainium2 NeuronCores — v2.

Measured-design notes (see exp/):
  - SWDGE dma_gather costs ~8.6ns/row of Q7 desc-gen; rotating 4 SWDGE
    queues gives ~2.8x. 1024 idxs/instruction is the validated max.
  - int16 gather indices span <32768 rows -> fp16 table [x(64)|p2] at 256B
    stride, 4 color ranges of 25088 rows. A host greedy colors nodes to
    balance per-dst in-edge colors (slot padding ~1.42x).
  - dst-sharded cores (snake over degree-sorted); per-(tile,color) K is the
    max across all 8 cores so the SPMD program is identical everywhere.
  - Phase 2 runs per group of tiles: gathers (color-major, ragged per-tile
    K), one PE matmul replicates p1b[dst] per column, DVE does
    gate/messages, equal-K-run tree adds reduce each tile's columns.
"""

import os
import sys

sys.path.insert(0, "/opt/trn_rl_repo")

import numpy as np

P = 128
NCORES = 8
D = 64
ELEM = 65            # fp16 payload per table row: x(64) + p2
FS = 128             # table row stride in fp16 elems (256B)
NCOLOR = 4
CPP = 32
IPC = 784
RSZ = CPP * IPC      # 25088 rows per color range
NPAD = NCOLOR * RSZ  # 100352
NUMG = 1024
CMAX = 256

LAST_RESULTS = None


def _ceil_to(a, m):
    return ((a + m - 1) // m) * m


class Plan:
    pass


# ---------------------------------------------------------------------------
# host prep (structure-only)
# ---------------------------------------------------------------------------

def _color_nodes(src, dst, N, rng):
    """Color nodes 0..3 balancing per-dst in-edge color counts.
    Chunked greedy sweeps (sum objective), then overflow-penalized sweeps."""
    cap = RSZ - 1
    deg = np.bincount(dst, minlength=N)
    tgt = np.ceil(np.concatenate([deg, [0]]) / NCOLOR).astype(np.int32)
    outdeg = np.bincount(src, minlength=N)
    mo = int(outdeg.max())
    # padded out-adjacency
    oo = np.argsort(src, kind="stable")
    dsort = dst[oo]
    ost = np.zeros(N + 1, np.int64)
    np.cumsum(np.bincount(src[oo], minlength=N), out=ost[1:])
    adj = np.zeros((N, mo), np.int64)
    msk = np.arange(mo)[None, :] < outdeg[:, None]
    adj[msk] = dsort
    adj[~msk] = N  # sentinel row in padded cnt

    color = rng.integers(0, NCOLOR, size=N).astype(np.int8)
    cnt = np.zeros((N + 1, NCOLOR), np.int32)
    np.add.at(cnt, (dst, color[src]), 1)
    CH = 512
    ar = np.arange(CH)
    for sweep in range(7):
        BIG = 0 if sweep < 3 else 1024
        perm = rng.permutation(N)
        for i0 in range(0, N, CH):
            nodes = perm[i0 : i0 + CH]
            nn = len(nodes)
            ds = adj[nodes]                      # [nn, mo]
            vm = msk[nodes]                      # [nn, mo]
            cn = cnt[ds]                         # [nn, mo, 4]
            cur = color[nodes]
            # remove own contribution from current color
            base = cn.copy()
            jj, ll = np.nonzero(vm)
            base[jj, ll, cur[jj]] -= 1
            if BIG:
                t = tgt[ds][:, :, None]
                sc = ((np.maximum(base + 1 - t, 0) * BIG + base + 1)
                      * vm[:, :, None]).sum(axis=1)
            else:
                sc = (base * vm[:, :, None]).sum(axis=1)
            best = np.argmin(sc, axis=1).astype(np.int8)
            better = sc[ar[:nn], best] < sc[ar[:nn], cur]
            chg = np.nonzero(better & (best != cur))[0]
            if len(chg) == 0:
                continue
            chn = nodes[chg]
            dsc = adj[chn][msk[chn]]
            np.subtract.at(cnt, (dsc, np.repeat(cur[chg], outdeg[chn])), 1)
            np.add.at(cnt, (dsc, np.repeat(best[chg], outdeg[chn])), 1)
            color[chn] = best[chg]
    # capacity repair
    sizes = np.bincount(color, minlength=NCOLOR)
    for c in range(NCOLOR):
        while sizes[c] > cap:
            idx = np.where(color == c)[0]
            tgtc = int(np.argmin(sizes))
            over = idx[: sizes[c] - cap]
            color[over] = tgtc
            sizes = np.bincount(color, minlength=NCOLOR)
    return color


def _prep(x, gate_w, gate_b, src, dst):
    x = np.asarray(x, dtype=np.float32)
    gate_w = np.asarray(gate_w, dtype=np.float32)
    gate_b = np.asarray(gate_b, dtype=np.float32)
    src = np.asarray(src).astype(np.int64)
    dst = np.asarray(dst).astype(np.int64)
    N = x.shape[0]
    E = src.shape[0]
    rng = np.random.default_rng(12345)

    deg = np.bincount(dst, minlength=N).astype(np.int64)
    norm = np.clip(deg, 1.0, None) ** -0.5

    color = _color_nodes(src, dst, N, rng)
    row_of = np.full(N, -1, dtype=np.int64)
    for c in range(NCOLOR):
        nodes_c = np.where(color == c)[0]
        row_of[nodes_c] = c * RSZ + 1 + np.arange(len(nodes_c))
    assert (row_of >= 0).all() and (row_of % RSZ > 0).all()

    xp = np.zeros((NPAD, D), dtype=np.float32)
    xp[row_of] = x

    order = np.argsort(-deg, kind="stable")
    n8 = _ceil_to(N, NCORES)
    order_p = np.concatenate([order, np.full(n8 - N, -1, dtype=np.int64)])
    blocks = order_p.reshape(-1, NCORES).copy()
    blocks[1::2] = blocks[1::2, ::-1]
    core_nodes = np.ascontiguousarray(blocks.T)
    npc = core_nodes.shape[1]
    NPC_PAD = _ceil_to(npc, P)
    TILES = NPC_PAD // P
    pad = np.full((NCORES, NPC_PAD - npc), -1, dtype=np.int64)
    core_nodes = np.concatenate([core_nodes, pad], axis=1)

    cnt = np.zeros((N, NCOLOR), dtype=np.int32)
    np.add.at(cnt, (dst, color[src]), 1)
    for c in range(NCORES):
        nodes = core_nodes[c]
        real = nodes >= 0
        cc = cnt[np.clip(nodes, 0, N - 1)] * real[:, None]
        key = np.lexsort((cc[:, 3], cc[:, 2], cc[:, 1], cc[:, 0], ~real))
        core_nodes[c] = nodes[key]

    cnt_cores = np.zeros((NCORES, NPC_PAD, NCOLOR), dtype=np.int32)
    for c in range(NCORES):
        nodes = core_nodes[c]
        real = nodes >= 0
        cnt_cores[c] = cnt[np.clip(nodes, 0, N - 1)] * real[:, None]
    Ktc = np.zeros((TILES, NCOLOR), dtype=np.int64)
    for t in range(TILES):
        Ktc[t] = np.maximum(
            cnt_cores[:, t * P : (t + 1) * P, :].max(axis=(0, 1)), 1
        )

    # groups of tiles, capped by total columns
    groups = []
    t0 = 0
    while t0 < TILES:
        t1 = t0 + 1
        cols = int(Ktc[t0].sum())
        while t1 < TILES and cols + int(Ktc[t1].sum()) <= CMAX:
            cols += int(Ktc[t1].sum())
            t1 += 1
        groups.append((t0, t1))
        t0 = t1
    ngrp = len(groups)

    pl = Plan()
    pl.N, pl.E = N, E
    pl.NPC_PAD, pl.TILES = NPC_PAD, TILES
    pl.core_nodes = core_nodes
    pl.groups = groups
    pl.Ktc = Ktc

    # column layout per group: for color c: for tile t: Ktc[t,c] columns
    # gathers: per (group, color) chunked at NUMG idxs, queue balanced
    grp_cols = []
    col_of_grp = [0]
    colmap = []  # per group: list over colors of list of (t, colbase_in_grp)
    gathers = []  # (grp, color, colbase_in_grp, ncols, qcol_off16, queue)
    qload = [0, 0, 0, 0]
    qcol = [0, 0, 0, 0]
    for g, (a, b) in enumerate(groups):
        cols = 0
        cm = []
        for cidx in range(NCOLOR):
            tl = []
            for t in range(a, b):
                tl.append((t, cols))
                cols += int(Ktc[t, cidx])
            cm.append(tl)
        colmap.append(cm)
        grp_cols.append(cols)
        col_of_grp.append(col_of_grp[-1] + cols)
        # gather chunks per color
        for cidx in range(NCOLOR):
            cstart = cm[cidx][0][1]
            cend = cm[cidx][-1][1] + int(Ktc[b - 1, cidx])
            g0 = cstart
            while g0 < cend:
                g1 = min(g0 + NUMG // P, cend)
                qn = int(np.argmin(qload))
                qload[qn] += g1 - g0
                gathers.append((g, cidx, g0, g1 - g0, qcol[qn], qn))
                qcol[qn] += (g1 - g0) * P // 16
                g0 = g1
    pl.grp_cols = grp_cols
    pl.col_of_grp = col_of_grp
    pl.colmap = colmap
    pl.gathers = gathers
    pl.CTOT = col_of_grp[-1]
    pl.IDX16 = max(qcol)

    # per-core streams
    ekey = dst * NCOLOR + color[src]
    e_order = np.argsort(ekey, kind="stable")
    src_sorted = src[e_order]
    cum = np.zeros(N * NCOLOR + 1, dtype=np.int64)
    np.cumsum(np.bincount(ekey, minlength=N * NCOLOR), out=cum[1:])

    wrep16 = np.broadcast_to(
        gate_w[0, D : 2 * D].astype(np.float16), (P, D)
    ).copy()
    wrep32 = np.broadcast_to(gate_w[0, 0:D], (P, D)).copy()
    b128 = np.full((P, 1), float(gate_b.reshape(-1)[0]), dtype=np.float32)

    # column -> (tile, color, k) tables (shared across cores)
    col_tile = np.zeros(pl.CTOT, dtype=np.int64)
    col_cidx = np.zeros(pl.CTOT, dtype=np.int64)
    col_k = np.zeros(pl.CTOT, dtype=np.int64)
    for g, (a, b) in enumerate(groups):
        base = col_of_grp[g]
        for cidx in range(NCOLOR):
            for (t, cb) in colmap[g][cidx]:
                K = int(Ktc[t, cidx])
                col_tile[base + cb : base + cb + K] = t
                col_cidx[base + cb : base + cb + K] = cidx
                col_k[base + cb : base + cb + K] = np.arange(K)

    in_maps = []
    for c in range(NCORES):
        nodes = core_nodes[c]
        nodes_cl = np.clip(nodes, 0, N - 1)
        real = nodes >= 0
        # vectorized per-column source rows
        lane_nodes = nodes_cl.reshape(TILES, P)
        lane_real = real.reshape(TILES, P)
        tcol = col_tile                                 # [CTOT]
        base_e = cum[lane_nodes[tcol] * NCOLOR + col_cidx[:, None]]
        cnt_e = (
            cum[lane_nodes[tcol] * NCOLOR + col_cidx[:, None] + 1] - base_e
        )
        has = lane_real[tcol] & (col_k[:, None] < cnt_e)
        e_idx = base_e + np.minimum(
            col_k[:, None], np.maximum(cnt_e - 1, 0)
        )
        s_nodes = src_sorted[e_idx]                     # [CTOT, P]
        rows = row_of[s_nodes] - col_cidx[:, None] * RSZ
        ids_all = np.where(has, rows, 0).astype(np.int16)  # [CTOT, P]
        npr = np.where(
            has, norm[s_nodes] * norm[lane_nodes[tcol]], 0.0
        ).T.astype(np.float16).copy()                   # [P, CTOT]
        ind = np.zeros((P, pl.CTOT), dtype=np.float16)
        ind[col_tile, np.arange(pl.CTOT)] = 1.0

        idx16 = np.zeros((P, pl.IDX16), dtype=np.int16)
        for (g, cidx, cb, nc_, io, qn) in gathers:
            c0 = col_of_grp[g] + cb
            flat = ids_all[c0 : c0 + nc_].reshape(-1)
            n16 = len(flat) // 16
            wrapped = flat.reshape(n16, 16).T
            pb = qn * 32
            idx16[pb : pb + 16, io : io + n16] = wrapped
            idx16[pb + 16 : pb + 32, io : io + n16] = wrapped

        xown = np.zeros((NPC_PAD, D), dtype=np.float32)
        xown[real] = x[nodes_cl[real]]

        in_maps.append(
            {
                "xp": xp,
                "wrep16": wrep16,
                "wrep32": wrep32,
                "b128": b128,
                "xown": xown,
                "idx16": idx16,
                "npr": npr,
                "ind": ind,
            }
        )
    return pl, in_maps


# ---------------------------------------------------------------------------
# numpy emulation (prep validation)
# ---------------------------------------------------------------------------

def emulate(pl, in_maps):
    outs = []
    for c in range(NCORES):
        mm = in_maps[c]
        tabx = mm["xp"].astype(np.float16)
        p2 = (
            tabx.astype(np.float32) @ mm["wrep16"][0].astype(np.float32)
        ).astype(np.float16)
        p1b = (mm["xown"] @ mm["wrep32"][0] + mm["b128"][0, 0]).astype(
            np.float32
        )
        z = np.zeros((pl.NPC_PAD, D), dtype=np.float32)
        for (g, cidx, cb, nc_, io, qn) in pl.gathers:
            n16 = nc_ * P // 16
            pb = qn * 32
            flat = mm["idx16"][pb : pb + 16, io : io + n16].T.reshape(-1)
            ids = flat.reshape(nc_, P).astype(np.int64) + cidx * RSZ
            c0 = pl.col_of_grp[g] + cb
            for j in range(nc_):
                gcol = c0 + j
                t = int(np.nonzero(mm["ind"][:, gcol])[0][0])
                rows = ids[j]
                xs = tabx[rows].astype(np.float32)
                p2s = p2[rows].astype(np.float32)
                lanes = np.arange(P) + t * P
                arg = np.float16(p2s + p1b[lanes]).astype(np.float32)
                ee = np.float16(
                    np.tanh(arg) * mm["npr"][:, gcol].astype(np.float32)
                )
                z[lanes] += np.float16(
                    xs * ee.astype(np.float32)[:, None]
                ).astype(np.float32)
        outs.append(z)
    return outs


# ---------------------------------------------------------------------------
# device program
# ---------------------------------------------------------------------------

def _dma_gather(nc, mybir, out_ap, in_ap, idxs_ap, num_idxs, queue_num):
    gp = nc.gpsimd
    from concourse.bass import exact_div

    stride_bytes_256 = exact_div(FS * mybir.dt.size(in_ap.dtype), 256)
    _in_ap = gp.lower_ap_dma(in_ap, for_custom_bir_dma=True)
    _idxs_ap = gp.lower_ap(idxs_ap)
    _out_ap = gp.lower_ap(out_ap)
    return gp.add_instruction(
        mybir.InstDMAGatherAnt(
            name=gp.bass.get_next_instruction_name(),
            ins=[*_in_ap, _idxs_ap, gp.lower_val_access(gp.to_reg(num_idxs))],
            outs=[_out_ap],
            transpose=False,
            num_idxs=num_idxs,
            elem_size=ELEM,
            stride_bytes_256=stride_bytes_256,
            gen_mode=0,
            single_packet=True,
            queue_num=queue_num,
            sbuf_tokens_per_rank=0,
            sbuf_free_dim_per_rank=0,
            sbuf_free_dim_pad_per_rank=0,
            sbuf_byte_offset=0,
        )
    )


def _build_nc(pl):
    import concourse.bacc as bacc
    import concourse.mybir as mybir
    import concourse.tile as tile
    from concourse.masks import make_identity

    f32 = mybir.dt.float32
    f16 = mybir.dt.float16
    i16 = mybir.dt.int16
    AF = mybir.ActivationFunctionType
    OP = mybir.AluOpType
    AX = mybir.AxisListType

    TILES, NPC_PAD = pl.TILES, pl.NPC_PAD

    nc = bacc.Bacc(
        "TRN2",
        target_bir_lowering=False,
        debug=False,
        num_devices=NCORES,
        num_swdge_queues=4,
    )
    xp_d = nc.dram_tensor("xp", [NPAD, D], f32, kind="ExternalInput")
    w16_d = nc.dram_tensor("wrep16", [P, D], f16, kind="ExternalInput")
    w32_d = nc.dram_tensor("wrep32", [P, D], f32, kind="ExternalInput")
    b128_d = nc.dram_tensor("b128", [P, 1], f32, kind="ExternalInput")
    xown_d = nc.dram_tensor("xown", [NPC_PAD, D], f32, kind="ExternalInput")
    idx_d = nc.dram_tensor("idx16", [P, pl.IDX16], i16, kind="ExternalInput")
    npr_d = nc.dram_tensor("npr", [P, pl.CTOT], f16, kind="ExternalInput")
    ind_d = nc.dram_tensor("ind", [P, pl.CTOT], f16, kind="ExternalInput")
    z_d = nc.dram_tensor("z", [NPC_PAD, D], f32, kind="ExternalOutput")
    tab_d = nc.dram_tensor("tab", [NPAD, FS], f16)

    with tile.TileContext(nc) as tc, nc.allow_low_precision("fp16 messages"):
        with tc.tile_pool(name="consts", bufs=1) as cpool:
            w16_sb = cpool.tile([P, D], f16)
            nc.sync.dma_start(out=w16_sb[:], in_=w16_d[:, :])
            w32_sb = cpool.tile([P, D], f32)
            nc.sync.dma_start(out=w32_sb[:], in_=w32_d[:, :])
            b128_sb = cpool.tile([P, 1], f32)
            nc.sync.dma_start(out=b128_sb[:], in_=b128_d[:, :])
            idx_sb = cpool.tile([P, pl.IDX16], i16)
            nc.sync.dma_start(out=idx_sb[:], in_=idx_d[:, :])
            npr_sb = cpool.tile([P, pl.CTOT], f16)
            nc.sync.dma_start(out=npr_sb[:], in_=npr_d[:, :])
            ind_sb = cpool.tile([P, pl.CTOT], f16)
            nc.sync.dma_start(out=ind_sb[:], in_=ind_d[:, :])
            p1bT = cpool.tile([P, P], f16)
            p1b_sb = cpool.tile([P, TILES], f32)

            # ---- phase 1 --------------------------------------------------
            with tc.tile_pool(name="rowb", bufs=1) as rpool:
                rowbuf = rpool.tile([P, IPC * ELEM], f16)
                rbv = rowbuf[:].rearrange("p (i f) -> p i f", f=ELEM)
                BC = 56
                xpv = xp_d[:, :].rearrange("(p i) f -> p i f", p=P)
                with tc.tile_pool(name="ph1a", bufs=2) as papool:
                    for c0 in range(0, IPC, BC):
                        cn = min(BC, IPC - c0)
                        nc.gpsimd.dma_start(
                            out=rbv[:, c0 : c0 + cn, 0:D],
                            in_=xpv[:, c0 : c0 + cn, :],
                        )
                        tmp = papool.tile([P, BC * D], f16, tag="tmp")
                        tv = tmp[:].rearrange("p (i f) -> p i f", f=D)
                        nc.vector.tensor_tensor(
                            out=tv[:, 0:cn, :],
                            in0=rbv[:, c0 : c0 + cn, 0:D],
                            in1=w16_sb[:]
                            .rearrange("p (o f) -> p o f", o=1)
                            .to_broadcast([P, cn, D]),
                            op=OP.mult,
                        )
                        nc.vector.tensor_reduce(
                            out=rbv[:, c0 : c0 + cn, D],
                            in_=tv[:, 0:cn, :],
                            axis=AX.X,
                            op=OP.add,
                        )
                tabv = tab_d[:, 0:ELEM].rearrange("(p i) f -> p i f", p=P)
                for c in range(NCOLOR):
                    nc.sync.dma_start(
                        out=tabv[c * CPP : (c + 1) * CPP, :, :],
                        in_=rbv[c * CPP : (c + 1) * CPP, :, :],
                    )

            # ---- phase 1b: p1b + PE transpose -----------------------------
            with tc.tile_pool(name="ph1", bufs=1) as p1pool:
                xo = p1pool.tile([P, TILES * D], f32, tag="xo")
                xov = xo[:].rearrange("p (t f) -> p t f", f=D)
                nc.sync.dma_start(
                    out=xov[:, :, :],
                    in_=xown_d[:, :].rearrange("(t p) f -> p t f", t=TILES),
                )
                tmp2 = p1pool.tile([P, TILES * D], f32, tag="tmp2")
                t2v = tmp2[:].rearrange("p (t f) -> p t f", f=D)
                nc.vector.tensor_tensor(
                    out=t2v[:, :, :],
                    in0=xov[:, :, :],
                    in1=w32_sb[:]
                    .rearrange("p (o f) -> p o f", o=1)
                    .to_broadcast([P, TILES, D]),
                    op=OP.mult,
                )
                red = p1pool.tile([P, TILES], f32, tag="red")
                nc.vector.tensor_reduce(
                    out=red[:], in_=t2v[:, :, :], axis=AX.X, op=OP.add
                )
                nc.vector.tensor_scalar(
                    out=p1b_sb[:], in0=red[:], scalar1=b128_sb[:, 0:1],
                    scalar2=None, op0=OP.add,
                )
                with tc.tile_pool(name="ps_t", bufs=1, space="PSUM") as ps_t:
                    ident = p1pool.tile([P, P], f32, tag="ident")
                    make_identity(nc, ident[:])
                    p1bT_ps = ps_t.tile([P, P], f32, tag="p1bt")
                    nc.tensor.transpose(
                        out=p1bT_ps[0:TILES, 0:P],
                        in_=p1b_sb[:, 0:TILES],
                        identity=ident[:],
                    )
                    nc.vector.tensor_copy(
                        out=p1bT[0:TILES, :], in_=p1bT_ps[0:TILES, 0:P]
                    )

            # ---- phase 2 --------------------------------------------------
            gidx = 0
            with (
                tc.tile_pool(name="ga", bufs=3) as gapool,
                tc.tile_pool(name="mm", bufs=1) as mpool,
                tc.tile_pool(name="sc", bufs=2) as spool,
                tc.tile_pool(name="ps", bufs=2, space="PSUM") as pspool,
            ):
                for g, (a, b) in enumerate(pl.groups):
                    G = b - a
                    C = pl.grp_cols[g]
                    cb0 = int(pl.col_of_grp[g])
                    ga = gapool.tile([P, C * ELEM], f16, tag="ga")
                    gav = ga[:].rearrange("p (c f) -> p c f", f=ELEM)
                    while gidx < len(pl.gathers) and pl.gathers[gidx][0] == g:
                        (_, cidx, cb, nc_, io, qn) = pl.gathers[gidx]
                        _dma_gather(
                            nc, mybir,
                            out_ap=gav[:, cb : cb + nc_, :],
                            in_ap=tab_d[cidx * RSZ : (cidx + 1) * RSZ, 0:ELEM],
                            idxs_ap=idx_sb[:, io : io + nc_ * P // 16],
                            num_idxs=nc_ * P,
                            queue_num=qn,
                        )
                        gidx += 1

                    pcols = pspool.tile([P, C], f32, tag="pcols")
                    nc.tensor.matmul(
                        out=pcols[:],
                        lhsT=p1bT[0:TILES, :],
                        rhs=ind_sb[0:TILES, cb0 : cb0 + C],
                    )
                    arg = spool.tile([P, C], f16, tag="arg")
                    nc.vector.tensor_tensor(
                        out=arg[:], in0=gav[:, :, D], in1=pcols[:], op=OP.add
                    )
                    tt = spool.tile([P, C], f16, tag="tt")
                    nc.scalar.activation(out=tt[:], in_=arg[:], func=AF.Tanh)
                    ee = spool.tile([P, C], f16, tag="ee")
                    nc.vector.tensor_tensor(
                        out=ee[:], in0=tt[:], in1=npr_sb[:, cb0 : cb0 + C],
                        op=OP.mult,
                    )
                    m = mpool.tile([P, C * D], f16, tag="m")
                    mv = m[:].rearrange("p (c f) -> p c f", f=D)
                    nc.vector.tensor_tensor(
                        out=mv[:, :, :],
                        in0=gav[:, :, 0:D],
                        in1=ee[:].rearrange("p (c o) -> p c o", o=1)
                        .to_broadcast([P, C, D]),
                        op=OP.mult,
                    )
                    zpart = spool.tile([P, NCOLOR * G * D], f16, tag="zp")
                    zpv = zpart[:].rearrange(
                        "p (r t f) -> p r t f", r=NCOLOR, f=D
                    )
                    for cidx in range(NCOLOR):
                        # equal-K runs of tiles in natural order
                        tl = pl.colmap[g][cidx]
                        i = 0
                        while i < len(tl):
                            t_i, cb_i = tl[i]
                            K = int(pl.Ktc[t_i, cidx])
                            j = i + 1
                            while j < len(tl) and int(
                                pl.Ktc[tl[j][0], cidx]
                            ) == K:
                                j += 1
                            RL = j - i
                            mseg = m[
                                :, cb_i * D : (cb_i + RL * K) * D
                            ].rearrange("p (t k f) -> p t k f", t=RL, f=D)
                            k = K
                            while k > 1:
                                h2 = 1 << (k.bit_length() - 1)
                                if h2 == k:
                                    h2 = k // 2
                                r = k - h2
                                nc.vector.tensor_tensor(
                                    out=mseg[:, :, 0:r, :],
                                    in0=mseg[:, :, 0:r, :],
                                    in1=mseg[:, :, h2:k, :],
                                    op=OP.add,
                                )
                                k = h2
                            nc.vector.tensor_copy(
                                out=zpv[:, cidx, t_i - a : t_i - a + RL, :],
                                in_=mseg[:, :, 0, :],
                            )
                            i = j
                    nc.vector.tensor_tensor(
                        out=zpv[:, 0, :, :], in0=zpv[:, 0, :, :],
                        in1=zpv[:, 1, :, :], op=OP.add,
                    )
                    nc.vector.tensor_tensor(
                        out=zpv[:, 2, :, :], in0=zpv[:, 2, :, :],
                        in1=zpv[:, 3, :, :], op=OP.add,
                    )
                    zt = spool.tile([P, G * D], f32, tag="zt")
                    nc.vector.tensor_tensor(
                        out=zt[:].rearrange("p (t f) -> p t f", f=D),
                        in0=zpv[:, 0, :, :], in1=zpv[:, 2, :, :], op=OP.add,
                    )
                    nc.sync.dma_start(
                        out=z_d[a * P : b * P, :].rearrange(
                            "(t p) f -> p t f", t=G
                        ),
                        in_=zt[:].rearrange("p (t f) -> p t f", f=D),
                    )
    nc.compile()
    return nc


_BUILD_CACHE = {}


def _assemble(pl, outs):
    z = np.zeros((pl.N, D), dtype=np.float32)
    for c in range(NCORES):
        nodes = pl.core_nodes[c]
        real = nodes >= 0
        z[nodes[real]] = outs[c][real]
    return z


def kernel(x, gate_w, gate_b, src, dst):
    global LAST_RESULTS
    pl, in_maps = _prep(x, gate_w, gate_b, src, dst)
    if os.environ.get("FAGCN_EMU"):
        return _assemble(pl, emulate(pl, in_maps))
    from concourse.bass_utils import run_bass_kernel_spmd

    key = (pl.N, pl.E, tuple(pl.grp_cols))
    nc = _BUILD_CACHE.get(key)
    if nc is None:
        nc = _build_nc(pl)
        _BUILD_CACHE[key] = nc
    res = run_bass_kernel_spmd(
        nc,
        in_maps,
        core_ids=list(range(NCORES)),
        trace=bool(int(os.environ.get("FAGCN_TRACE", "0"))),
    )
    LAST_RESULTS = res
    outs = [r["z"] for r in res.results]
    return _assemble(pl, outs)
